# revision 2
# baseline (speedup 1.0000x reference)
"""RAFT-style CorrBlock kernel for Trainium2 (8 NeuronCores, Bass/Tile). v2: bf16.

Full inputs: fmap1 [2,256,64,64], fmap2 [2,256,64,64], centroids_coords [2,2,64,64].
Output: [2, 324, 64, 64] f32.

Sharding: data-parallel over the B*H1*W1 query-pixel axis. Core c handles batch
c//4, query pixels (c%4)*1024 .. +1024.

v2 changes vs baseline:
  - f1/f2 cast to bf16 on host; matmuls run bf16 (1 cycle/row vs 4 for fp32).
  - corr pyramid, slab, bands, masks, weights, feats all bf16: halves the
    DRAM slab round-trip and doubles DVE throughput for the combine.
  - pool-y moved from gpsimd to vector (gpsimd reserved for indirect gathers).
  - output written bf16, cast to f32 on host.
"""

import numpy as np
import os

import ml_dtypes

import concourse.bass as bass
import concourse.bacc as bacc
import concourse.mybir as mybir
import concourse.tile as tile
from concourse.bass_utils import run_bass_kernel_spmd

f32 = mybir.dt.float32
bf16 = mybir.dt.bfloat16
i32 = mybir.dt.int32
OP = mybir.AluOpType

P = 128
C = 256
HW = 4096          # h2*w2 at level 0
NPIX = 1024        # query pixels per core
NG = NPIX // P     # 8 groups of 128 pixels
NLVL = 4
S = 9              # sample window side (2*RADIUS+1)
PS = 10            # patch side
W_L = [64, 32, 16, 8]
HW_L = [w * w for w in W_L]           # 4096, 1024, 256, 64
B_L = [9 * w + PS for w in W_L]       # band length: 586, 298, 154, 82
BASE_L = [0]
for _l in range(1, NLVL):
    BASE_L.append(BASE_L[-1] + NPIX * HW_L[_l - 1])
TOT = BASE_L[-1] + NPIX * HW_L[-1]    # 1024*5440
G = 1024                              # zeroed guard elements at both slab ends
NT = G + TOT + G
FEAT = NLVL * S * S                   # 324


def _ap_view(t_ap, offset, dims):
    """Arbitrary strided view of a tile AP: dims = [[step, count], ...] free dims."""
    return bass.AP(t_ap.tensor, t_ap.offset + offset, [list(t_ap.ap[0])] + dims)


def build_bass():
    nc = bacc.Bacc("TRN2", target_bir_lowering=False, debug=False)

    f1_d = nc.dram_tensor("f1", [C, NPIX], bf16, kind="ExternalInput")
    f2_d = nc.dram_tensor("f2", [C, HW], bf16, kind="ExternalInput")
    ccx_d = nc.dram_tensor("ccx", [P, NG], f32, kind="ExternalInput")
    ccy_d = nc.dram_tensor("ccy", [P, NG], f32, kind="ExternalInput")
    out_d = nc.dram_tensor("out", [NPIX, FEAT], bf16, kind="ExternalOutput")
    slab_d = nc.dram_tensor("slab", [NT], bf16)  # Internal scratch

    with tile.TileContext(nc) as tc:
        with (
            tc.tile_pool(name="persist", bufs=1) as pp,
            tc.tile_pool(name="grp", bufs=2) as pg,
            tc.tile_pool(name="psum", bufs=8, space="PSUM") as ps,
            tc.tile_pool(name="post", bufs=1) as po,
        ):
            # ---- guard zero-fill ----
            zt = pp.tile([1, G], bf16, tag="zt")
            nc.vector.memset(zt[:], 0.0)
            nc.sync.dma_start(slab_d.ap()[0:G][None, :], zt[:])
            nc.sync.dma_start(slab_d.ap()[NT - G:NT][None, :], zt[:])

            # ---- input loads ----
            f1t = []
            f2t = []
            for k in range(2):
                t1 = pp.tile([P, NPIX], bf16, tag=f"f1_{k}")
                nc.sync.dma_start(t1[:], f1_d.ap()[k * P:(k + 1) * P, :])
                f1t.append(t1)
                t2 = pp.tile([P, HW], bf16, tag=f"f2_{k}")
                nc.sync.dma_start(t2[:], f2_d.ap()[k * P:(k + 1) * P, :])
                f2t.append(t2)
            ccx = pp.tile([P, NG], f32, tag="ccx")
            ccy = pp.tile([P, NG], f32, tag="ccy")
            nc.sync.dma_start(ccx[:], ccx_d.ap())
            nc.sync.dma_start(ccy[:], ccy_d.ap())

            REP = int(os.environ.get("K_REPEAT", "1"))
            for _rep in range(REP):
                # ---- pool f2 spatially (sums, not means; scale folded into weights) ----
                # f2l[l][k] : [128, HW_L[l]] viewed as [H_l, W_l] row-major
                f2l = [f2t]
                for l in range(1, NLVL):
                    w_in = W_L[l - 1]
                    w_out = W_L[l]
                    cur = []
                    for k in range(2):
                        src = f2l[l - 1][k]
                        tmpx = pg.tile([P, w_in * w_out], bf16, tag=f"f2pool_tmp_{l}")
                        # pool x: tmpx[y, x'] = src[y, 2x'] + src[y, 2x'+1]
                        nc.vector.tensor_tensor(
                            out=tmpx[:],
                            in0=_ap_view(src[:], 0, [[w_in, w_in], [2, w_out]]),
                            in1=_ap_view(src[:], 1, [[w_in, w_in], [2, w_out]]),
                            op=OP.add,
                        )
                        dst = pp.tile([P, w_out * w_out], bf16, tag=f"f2l_{l}_{k}")
                        # pool y: dst[y', x'] = tmpx[2y'] + tmpx[2y'+1]
                        nc.vector.tensor_tensor(
                            out=dst[:],
                            in0=_ap_view(tmpx[:], 0, [[2 * w_out, w_out], [1, w_out]]),
                            in1=_ap_view(tmpx[:], w_out, [[2 * w_out, w_out], [1, w_out]]),
                            op=OP.add,
                        )
                        cur.append(dst)
                    f2l.append(cur)

                # ---- per-level index / weight / mask precompute ----
                # iota over patch coordinate k (0..9), replicated per group
                kvi = pp.tile([P, NG * PS], i32, tag="kvi")
                nc.gpsimd.iota(kvi[:], pattern=[[0, NG], [1, PS]], base=0, channel_multiplier=0)
                kvf = pp.tile([P, NG * PS], f32, tag="kvf")
                nc.vector.tensor_copy(out=kvf[:], in_=kvi[:])
                # pixel index n = g*128 + p as f32 (iota steps must fit int16)
                npix_i = pp.tile([P, NG], i32, tag="npix_i")
                nc.gpsimd.iota(npix_i[:], pattern=[[P, NG]], base=0, channel_multiplier=1)
                npix_f = pp.tile([P, NG], f32, tag="npix_f")
                nc.vector.tensor_copy(out=npix_f[:], in_=npix_i[:])

                idx_l = []
                w_l = []     # [w00, w01, w10, w11] per level, each [P, NG] bf16
                m_l = []     # [P, NG*100] patch validity masks (bf16)
                for l in range(NLVL):
                    wl = W_L[l]
                    inv = 1.0 / (1 << l)
                    sc = 1.0 / (16.0 * (4.0 ** l))

                    xs = pg.tile([P, NG], f32, tag="xs")
                    ys = pg.tile([P, NG], f32, tag="ys")
                    nc.vector.tensor_scalar_mul(xs[:], ccx[:], inv)
                    nc.vector.tensor_scalar_mul(ys[:], ccy[:], inv)

                    def floor_of(v, nm):
                        ti = pg.tile([P, NG], i32, tag=f"fl_i_{nm}")
                        nc.vector.tensor_copy(out=ti[:], in_=v[:])
                        tf = pg.tile([P, NG], f32, tag=f"fl_f_{nm}")
                        nc.vector.tensor_copy(out=tf[:], in_=ti[:])
                        gt = pg.tile([P, NG], f32, tag=f"fl_g_{nm}")
                        nc.vector.tensor_tensor(out=gt[:], in0=tf[:], in1=v[:], op=OP.is_gt)
                        fl = pg.tile([P, NG], f32, tag=f"fl_o_{nm}")
                        nc.vector.tensor_tensor(out=fl[:], in0=tf[:], in1=gt[:], op=OP.subtract)
                        return fl

                    x0 = floor_of(xs, "x")
                    y0 = floor_of(ys, "y")

                    fx = pg.tile([P, NG], f32, tag="fx")
                    fy = pg.tile([P, NG], f32, tag="fy")
                    nc.vector.tensor_tensor(out=fx[:], in0=xs[:], in1=x0[:], op=OP.subtract)
                    nc.vector.tensor_tensor(out=fy[:], in0=ys[:], in1=y0[:], op=OP.subtract)

                    # weights: w_ab = wy_a * wx_b * sc ; wx1 = fx, wx0 = 1-fx
                    wy0s = pg.tile([P, NG], f32, tag="wy0s")
                    wy1s = pg.tile([P, NG], f32, tag="wy1s")
                    # wy0*sc = (fy*-sc)+sc ; wy1*sc = fy*sc
                    nc.vector.tensor_scalar(wy0s[:], fy[:], -sc, sc, OP.mult, OP.add)
                    nc.vector.tensor_scalar_mul(wy1s[:], fy[:], sc)
                    wx0 = pg.tile([P, NG], f32, tag="wx0")
                    nc.vector.tensor_scalar(wx0[:], fx[:], -1.0, 1.0, OP.mult, OP.add)
                    ws = []
                    for a, wya in ((0, wy0s), (1, wy1s)):
                        for b, wxb in ((0, wx0), (1, fx)):
                            wt = pp.tile([P, NG], bf16, tag=f"w{a}{b}_{l}")
                            nc.vector.tensor_tensor(out=wt[:], in0=wya[:], in1=wxb[:], op=OP.mult)
                            ws.append(wt)
                    w_l.append(ws)

                    # band start index: n*HW_l + (y0-4)*W_l + (x0-4) + BASE_l
                    t1 = pg.tile([P, NG], f32, tag="idx_t1")
                    nc.vector.scalar_tensor_tensor(
                        out=t1[:], in0=y0[:], scalar=float(wl), in1=x0[:],
                        op0=OP.mult, op1=OP.add,
                    )
                    t2 = pg.tile([P, NG], f32, tag="idx_t2")
                    nc.vector.scalar_tensor_tensor(
                        out=t2[:], in0=npix_f[:], scalar=float(HW_L[l]), in1=t1[:],
                        op0=OP.mult, op1=OP.add,
                    )
                    t3 = pg.tile([P, NG], f32, tag="idx_t3")
                    nc.vector.tensor_scalar_add(t3[:], t2[:], float(BASE_L[l] - 4 * wl - 4))
                    ii = pp.tile([P, NG], i32, tag=f"idx_{l}")
                    nc.vector.tensor_copy(out=ii[:], in_=t3[:])
                    idx_l.append(ii)

                    # row/col validity: valid iff 4-k <= c0 <= H+3-k  (c0=y0 or x0)
                    def valid(c0, lim, nm):
                        # t = c0 + k   (broadcast c0 over k)
                        tt = pg.tile([P, NG * PS], f32, tag=f"v_t_{nm}")
                        nc.vector.tensor_tensor(
                            out=tt[:].rearrange("p (g k) -> p g k", k=PS),
                            in0=kvf[:].rearrange("p (g k) -> p g k", k=PS),
                            in1=c0[:, :, None].to_broadcast([P, NG, PS]),
                            op=OP.add,
                        )
                        c1 = pg.tile([P, NG * PS], f32, tag=f"v_c_{nm}")
                        nc.vector.tensor_scalar(c1[:], tt[:], 4.0, None, OP.is_ge)
                        vv = pg.tile([P, NG * PS], f32, tag=f"v_o_{nm}")
                        nc.vector.scalar_tensor_tensor(
                            out=vv[:], in0=tt[:], scalar=float(lim + 3), in1=c1[:],
                            op0=OP.is_le, op1=OP.mult,
                        )
                        return vv

                    rv = valid(y0, wl, "r")
                    cv = valid(x0, wl, "c")
                    mm = pp.tile([P, NG * PS * PS], bf16, tag=f"m_{l}")
                    nc.vector.tensor_tensor(
                        out=mm[:].rearrange("p (g a b) -> p g a b", a=PS, b=PS),
                        in0=rv[:].rearrange("p (g k) -> p g k", k=PS)[:, :, :, None]
                            .to_broadcast([P, NG, PS, PS]),
                        in1=cv[:].rearrange("p (g k) -> p g k", k=PS)[:, :, None, :]
                            .to_broadcast([P, NG, PS, PS]),
                        op=OP.mult,
                    )
                    m_l.append(mm)

                # ---- per-group: matmuls -> PSUM -> SBUF -> slab writes ----
                _skip_mm = os.environ.get("K_SKIP_MM") == "1"
                _skip_post = os.environ.get("K_SKIP_POST") == "1"
                ncopy = 0
                for g in range(NG) if not _skip_mm else []:
                    corr = [
                        pg.tile([P, HW_L[l]], bf16, tag=f"corr_{l}", name=f"corr_{l}_{g}_{_rep}")
                        for l in range(NLVL)
                    ]
                    for l in range(NLVL):
                        rhs_src = f2l[l]
                        hwl = HW_L[l]
                        nfree = min(512, hwl)
                        for n in range(hwl // nfree):
                            pt = ps.tile([P, 512], f32, tag="mm")
                            for k in range(2):
                                nc.tensor.matmul(
                                    out=pt[:, :nfree],
                                    lhsT=f1t[k][:, g * P:(g + 1) * P],
                                    rhs=rhs_src[k][:, n * nfree:(n + 1) * nfree],
                                    start=(k == 0),
                                    stop=(k == 1),
                                )
                            dst = corr[l][:, n * nfree:(n + 1) * nfree]
                            if ncopy % 2 == 0:
                                nc.scalar.copy(out=dst, in_=pt[:, :nfree])
                            else:
                                nc.vector.tensor_copy(out=dst, in_=pt[:, :nfree])
                            ncopy += 1
                    for l in range(NLVL):
                        ofs = G + BASE_L[l] + g * P * HW_L[l]
                        nc.sync.dma_start(
                            slab_d.ap()[ofs:ofs + P * HW_L[l]].rearrange("(p f) -> p f", f=HW_L[l]),
                            corr[l][:],
                        )

                # ---- band gathers + mask + bilinear combine ----
                feats = po.tile([P, NG * FEAT], bf16, tag="feats")
                if _skip_post:
                    nc.vector.memset(feats[:], 0.0)
                for l in range(NLVL) if not _skip_post else []:
                    bl = B_L[l]
                    wl = W_L[l]
                    band = po.tile([P, NG * bl], bf16, tag=f"band_{l}")
                    # HW DGE only honors one offset per partition -> one gather per group
                    if os.environ.get("K_SKIP_GATHER") == "1":
                        nc.vector.memset(band[:], 0.0)
                    else:
                        for g in range(NG):
                            nc.gpsimd.indirect_dma_start(
                                out=band[:, g * bl:(g + 1) * bl],
                                out_offset=None,
                                in_=slab_d.ap()[:, None],
                                in_offset=bass.IndirectOffsetOnAxis(ap=idx_l[l][:, g:g + 1], axis=0),
                                element_offset=G,
                            )
                    if os.environ.get("K_SKIP_CMB") == "1":
                        continue
                    # masked 10x10 patches, contiguous [P, NG*100]
                    pm = po.tile([P, NG * PS * PS], bf16, tag=f"pm_{l}")
                    nc.vector.tensor_tensor(
                        out=pm[:].rearrange("p (g a b) -> p g a b", a=PS, b=PS),
                        in0=_ap_view(band[:], 0, [[bl, NG], [wl, PS], [1, PS]]),
                        in1=m_l[l][:].rearrange("p (g a b) -> p g a b", a=PS, b=PS),
                        op=OP.mult,
                    )
                    # reference: sample (i, j) is at x = cc_x + (i-4), y = cc_y + (j-4),
                    # so out_tap[i, j] = sum_ab w_ab * patch[y=j+a, x=i+b]
                    ov = _ap_view(feats[:], l * S * S, [[FEAT, NG], [S, S], [1, S]])
                    for t, (a, b) in enumerate(((0, 0), (0, 1), (1, 0), (1, 1))):
                        pv = _ap_view(pm[:], a * PS + b, [[PS * PS, NG], [1, S], [PS, S]])
                        wb = w_l[l][t][:, :, None, None].to_broadcast([P, NG, S, S])
                        if t == 0:
                            nc.vector.tensor_tensor(out=ov, in0=pv, in1=wb, op=OP.mult)
                        else:
                            tmp = po.tile([P, NG * S * S], bf16, tag=f"cmb_tmp")
                            tv = tmp[:].rearrange("p (g a b) -> p g a b", a=S, b=S)
                            nc.vector.tensor_tensor(out=tv, in0=pv, in1=wb, op=OP.mult)
                            nc.vector.tensor_tensor(out=ov, in0=ov, in1=tv, op=OP.add)

                nc.sync.dma_start(
                    out_d.ap().rearrange("(g p) f -> p g f", p=P),
                    feats[:].rearrange("p (g f) -> p g f", f=FEAT),
                )

    nc.compile()
    return nc


_NC = None


def _get_nc():
    global _NC
    if _NC is None:
        _NC = build_bass()
    return _NC


def make_in_maps(fmap1, fmap2, centroids_coords):
    in_maps = []
    for core in range(8):
        bi, chunk = divmod(core, 4)
        m0 = chunk * NPIX
        f1 = np.ascontiguousarray(
            fmap1[bi].reshape(C, HW)[:, m0:m0 + NPIX]).astype(ml_dtypes.bfloat16)
        f2 = np.ascontiguousarray(fmap2[bi].reshape(C, HW)).astype(ml_dtypes.bfloat16)
        cc = centroids_coords[bi].reshape(2, HW)[:, m0:m0 + NPIX]
        ccx = np.ascontiguousarray(cc[0].reshape(NG, P).T, dtype=np.float32)  # [p, g]
        ccy = np.ascontiguousarray(cc[1].reshape(NG, P).T, dtype=np.float32)
        in_maps.append({"f1": f1, "f2": f2, "ccx": ccx, "ccy": ccy})
    return in_maps


def assemble(outs):
    """outs: list of 8 arrays [1024, 324] -> [2, 324, 64, 64]"""
    full = np.empty((2, FEAT, 64, 64), dtype=np.float32)
    for bi in range(2):
        feats = np.concatenate(
            [np.asarray(outs[bi * 4 + c]).astype(np.float32) for c in range(4)], axis=0)
        full[bi] = feats.reshape(64, 64, FEAT).transpose(2, 0, 1)
    return full


def kernel(fmap1, fmap2, centroids_coords, trace=False):
    nc = _get_nc()
    in_maps = make_in_maps(fmap1, fmap2, centroids_coords)
    try:
        res = run_bass_kernel_spmd(nc, in_maps, core_ids=list(range(8)), trace=trace)
    except ModuleNotFoundError:
        res = run_bass_kernel_spmd(nc, in_maps, core_ids=list(range(8)), trace=False)
    out = assemble([r["out"] for r in res.results])
    if trace:
        kernel.last_result = res
    return out


# revision 3
# speedup vs baseline: 1.0606x; 1.0606x over previous
"""RAFT-style CorrBlock kernel for Trainium2 (8 NeuronCores, Bass/Tile). v2: bf16.

Full inputs: fmap1 [2,256,64,64], fmap2 [2,256,64,64], centroids_coords [2,2,64,64].
Output: [2, 324, 64, 64] f32.

Sharding: data-parallel over the B*H1*W1 query-pixel axis. Core c handles batch
c//4, query pixels (c%4)*1024 .. +1024.

v2 changes vs baseline:
  - f1/f2 cast to bf16 on host; matmuls run bf16 (1 cycle/row vs 4 for fp32).
  - corr pyramid, slab, bands, masks, weights, feats all bf16: halves the
    DRAM slab round-trip and doubles DVE throughput for the combine.
  - pool-y moved from gpsimd to vector (gpsimd reserved for indirect gathers).
  - output written bf16, cast to f32 on host.
"""

import numpy as np
import os

import ml_dtypes

import concourse.bass as bass
import concourse.bacc as bacc
import concourse.mybir as mybir
import concourse.tile as tile
from concourse.bass_utils import run_bass_kernel_spmd

f32 = mybir.dt.float32
bf16 = mybir.dt.bfloat16
i32 = mybir.dt.int32
OP = mybir.AluOpType

P = 128
C = 256
HW = 4096          # h2*w2 at level 0
NPIX = 1024        # query pixels per core
NG = NPIX // P     # 8 groups of 128 pixels
NLVL = 4
S = 9              # sample window side (2*RADIUS+1)
PS = 10            # patch side
W_L = [64, 32, 16, 8]
HW_L = [w * w for w in W_L]           # 4096, 1024, 256, 64
B_L = [9 * w + PS for w in W_L]       # band length: 586, 298, 154, 82
BASE_L = [0]
for _l in range(1, NLVL):
    BASE_L.append(BASE_L[-1] + NPIX * HW_L[_l - 1])
TOT = BASE_L[-1] + NPIX * HW_L[-1]    # 1024*5440
G = 1024                              # zeroed guard elements at both slab ends
NT = G + TOT + G
FEAT = NLVL * S * S                   # 324


def _ap_view(t_ap, offset, dims):
    """Arbitrary strided view of a tile AP: dims = [[step, count], ...] free dims."""
    return bass.AP(t_ap.tensor, t_ap.offset + offset, [list(t_ap.ap[0])] + dims)


def build_bass():
    nc = bacc.Bacc("TRN2", target_bir_lowering=False, debug=False)

    f1_d = nc.dram_tensor("f1", [C, NPIX], bf16, kind="ExternalInput")
    f2_d = nc.dram_tensor("f2", [C, HW], bf16, kind="ExternalInput")
    ccx_d = nc.dram_tensor("ccx", [P, NG], f32, kind="ExternalInput")
    ccy_d = nc.dram_tensor("ccy", [P, NG], f32, kind="ExternalInput")
    out_d = nc.dram_tensor("out", [NPIX, FEAT], bf16, kind="ExternalOutput")
    slab_d = nc.dram_tensor("slab", [NT], bf16)  # Internal scratch

    with tile.TileContext(nc) as tc:
        with (
            tc.tile_pool(name="persist", bufs=1) as pp,
            tc.tile_pool(name="grp", bufs=2) as pg,
            tc.tile_pool(name="psum", bufs=8, space="PSUM") as ps,
            tc.tile_pool(name="post", bufs=1) as po,
        ):
            # ---- guard zero-fill ----
            zt = pp.tile([1, G], bf16, tag="zt")
            nc.vector.memset(zt[:], 0.0)
            nc.sync.dma_start(slab_d.ap()[0:G][None, :], zt[:])
            nc.sync.dma_start(slab_d.ap()[NT - G:NT][None, :], zt[:])

            # ---- input loads ----
            f1t = []
            f2t = []
            for k in range(2):
                t1 = pp.tile([P, NPIX], bf16, tag=f"f1_{k}")
                nc.sync.dma_start(t1[:], f1_d.ap()[k * P:(k + 1) * P, :])
                f1t.append(t1)
                t2 = pp.tile([P, HW], bf16, tag=f"f2_{k}")
                nc.sync.dma_start(t2[:], f2_d.ap()[k * P:(k + 1) * P, :])
                f2t.append(t2)
            ccx = pp.tile([P, NG], f32, tag="ccx")
            ccy = pp.tile([P, NG], f32, tag="ccy")
            nc.sync.dma_start(ccx[:], ccx_d.ap())
            nc.sync.dma_start(ccy[:], ccy_d.ap())

            REP = int(os.environ.get("K_REPEAT", "1"))
            for _rep in range(REP):
                # ---- pool f2 spatially (sums, not means; scale folded into weights) ----
                # f2l[l][k] : [128, HW_L[l]] viewed as [H_l, W_l] row-major
                f2l = [f2t]
                for l in range(1, NLVL):
                    w_in = W_L[l - 1]
                    w_out = W_L[l]
                    cur = []
                    for k in range(2):
                        src = f2l[l - 1][k]
                        tmpx = pg.tile([P, w_in * w_out], bf16, tag=f"f2pool_tmp_{l}")
                        # pool x: tmpx[y, x'] = src[y, 2x'] + src[y, 2x'+1]
                        nc.vector.tensor_tensor(
                            out=tmpx[:],
                            in0=_ap_view(src[:], 0, [[w_in, w_in], [2, w_out]]),
                            in1=_ap_view(src[:], 1, [[w_in, w_in], [2, w_out]]),
                            op=OP.add,
                        )
                        dst = pp.tile([P, w_out * w_out], bf16, tag=f"f2l_{l}_{k}")
                        # pool y: dst[y', x'] = tmpx[2y'] + tmpx[2y'+1]
                        nc.vector.tensor_tensor(
                            out=dst[:],
                            in0=_ap_view(tmpx[:], 0, [[2 * w_out, w_out], [1, w_out]]),
                            in1=_ap_view(tmpx[:], w_out, [[2 * w_out, w_out], [1, w_out]]),
                            op=OP.add,
                        )
                        cur.append(dst)
                    f2l.append(cur)

                # ---- per-level index / weight / mask precompute ----
                # iota over patch coordinate k (0..9), replicated per group
                kvi = pp.tile([P, NG * PS], i32, tag="kvi")
                nc.gpsimd.iota(kvi[:], pattern=[[0, NG], [1, PS]], base=0, channel_multiplier=0)
                kvf = pp.tile([P, NG * PS], f32, tag="kvf")
                nc.vector.tensor_copy(out=kvf[:], in_=kvi[:])
                # pixel index n = g*128 + p as f32 (iota steps must fit int16)
                npix_i = pp.tile([P, NG], i32, tag="npix_i")
                nc.gpsimd.iota(npix_i[:], pattern=[[P, NG]], base=0, channel_multiplier=1)
                npix_f = pp.tile([P, NG], f32, tag="npix_f")
                nc.vector.tensor_copy(out=npix_f[:], in_=npix_i[:])

                idx_l = []
                w_l = []     # [w00, w01, w10, w11] per level, each [P, NG] bf16
                m_l = []     # [P, NG*100] patch validity masks (bf16)
                for l in range(NLVL):
                    wl = W_L[l]
                    inv = 1.0 / (1 << l)
                    sc = 1.0 / (16.0 * (4.0 ** l))

                    xs = pg.tile([P, NG], f32, tag="xs")
                    ys = pg.tile([P, NG], f32, tag="ys")
                    nc.vector.tensor_scalar_mul(xs[:], ccx[:], inv)
                    nc.vector.tensor_scalar_mul(ys[:], ccy[:], inv)

                    def floor_of(v, nm):
                        ti = pg.tile([P, NG], i32, tag=f"fl_i_{nm}")
                        nc.vector.tensor_copy(out=ti[:], in_=v[:])
                        tf = pg.tile([P, NG], f32, tag=f"fl_f_{nm}")
                        nc.vector.tensor_copy(out=tf[:], in_=ti[:])
                        gt = pg.tile([P, NG], f32, tag=f"fl_g_{nm}")
                        nc.vector.tensor_tensor(out=gt[:], in0=tf[:], in1=v[:], op=OP.is_gt)
                        fl = pg.tile([P, NG], f32, tag=f"fl_o_{nm}")
                        nc.vector.tensor_tensor(out=fl[:], in0=tf[:], in1=gt[:], op=OP.subtract)
                        return fl

                    x0 = floor_of(xs, "x")
                    y0 = floor_of(ys, "y")

                    fx = pg.tile([P, NG], f32, tag="fx")
                    fy = pg.tile([P, NG], f32, tag="fy")
                    nc.vector.tensor_tensor(out=fx[:], in0=xs[:], in1=x0[:], op=OP.subtract)
                    nc.vector.tensor_tensor(out=fy[:], in0=ys[:], in1=y0[:], op=OP.subtract)

                    # weights: w_ab = wy_a * wx_b * sc ; wx1 = fx, wx0 = 1-fx
                    wy0s = pg.tile([P, NG], f32, tag="wy0s")
                    wy1s = pg.tile([P, NG], f32, tag="wy1s")
                    # wy0*sc = (fy*-sc)+sc ; wy1*sc = fy*sc
                    nc.vector.tensor_scalar(wy0s[:], fy[:], -sc, sc, OP.mult, OP.add)
                    nc.vector.tensor_scalar_mul(wy1s[:], fy[:], sc)
                    wx0 = pg.tile([P, NG], f32, tag="wx0")
                    nc.vector.tensor_scalar(wx0[:], fx[:], -1.0, 1.0, OP.mult, OP.add)
                    ws = []
                    for a, wya in ((0, wy0s), (1, wy1s)):
                        for b, wxb in ((0, wx0), (1, fx)):
                            wt = pp.tile([P, NG], bf16, tag=f"w{a}{b}_{l}")
                            nc.vector.tensor_tensor(out=wt[:], in0=wya[:], in1=wxb[:], op=OP.mult)
                            ws.append(wt)
                    w_l.append(ws)

                    # band start index: n*HW_l + (y0-4)*W_l + (x0-4) + BASE_l
                    t1 = pg.tile([P, NG], f32, tag="idx_t1")
                    nc.vector.scalar_tensor_tensor(
                        out=t1[:], in0=y0[:], scalar=float(wl), in1=x0[:],
                        op0=OP.mult, op1=OP.add,
                    )
                    t2 = pg.tile([P, NG], f32, tag="idx_t2")
                    nc.vector.scalar_tensor_tensor(
                        out=t2[:], in0=npix_f[:], scalar=float(HW_L[l]), in1=t1[:],
                        op0=OP.mult, op1=OP.add,
                    )
                    t3 = pg.tile([P, NG], f32, tag="idx_t3")
                    nc.vector.tensor_scalar_add(t3[:], t2[:], float(BASE_L[l] - 4 * wl - 4))
                    ii = pp.tile([P, NG], i32, tag=f"idx_{l}")
                    nc.vector.tensor_copy(out=ii[:], in_=t3[:])
                    idx_l.append(ii)

                    # row/col validity: valid iff 4-k <= c0 <= H+3-k  (c0=y0 or x0)
                    def valid(c0, lim, nm):
                        # t = c0 + k   (broadcast c0 over k)
                        tt = pg.tile([P, NG * PS], f32, tag=f"v_t_{nm}")
                        nc.vector.tensor_tensor(
                            out=tt[:].rearrange("p (g k) -> p g k", k=PS),
                            in0=kvf[:].rearrange("p (g k) -> p g k", k=PS),
                            in1=c0[:, :, None].to_broadcast([P, NG, PS]),
                            op=OP.add,
                        )
                        c1 = pg.tile([P, NG * PS], f32, tag=f"v_c_{nm}")
                        nc.vector.tensor_scalar(c1[:], tt[:], 4.0, None, OP.is_ge)
                        vv = pg.tile([P, NG * PS], f32, tag=f"v_o_{nm}")
                        nc.vector.scalar_tensor_tensor(
                            out=vv[:], in0=tt[:], scalar=float(lim + 3), in1=c1[:],
                            op0=OP.is_le, op1=OP.mult,
                        )
                        return vv

                    rv = valid(y0, wl, "r")
                    cv = valid(x0, wl, "c")
                    mm = pp.tile([P, NG * PS * PS], bf16, tag=f"m_{l}")
                    nc.vector.tensor_tensor(
                        out=mm[:].rearrange("p (g a b) -> p g a b", a=PS, b=PS),
                        in0=rv[:].rearrange("p (g k) -> p g k", k=PS)[:, :, :, None]
                            .to_broadcast([P, NG, PS, PS]),
                        in1=cv[:].rearrange("p (g k) -> p g k", k=PS)[:, :, None, :]
                            .to_broadcast([P, NG, PS, PS]),
                        op=OP.mult,
                    )
                    m_l.append(mm)

                # ---- per-group: matmuls -> PSUM -> SBUF -> slab writes ----
                _skip_mm = os.environ.get("K_SKIP_MM") == "1"
                _skip_post = os.environ.get("K_SKIP_POST") == "1"
                _cmod = int(os.environ.get("K_CMOD", "3"))
                ncopy = 0
                for g in range(NG) if not _skip_mm else []:
                    corr = [
                        pg.tile([P, HW_L[l]], bf16, tag=f"corr_{l}", name=f"corr_{l}_{g}_{_rep}")
                        for l in range(NLVL)
                    ]
                    for l in range(NLVL):
                        rhs_src = f2l[l]
                        hwl = HW_L[l]
                        nfree = min(512, hwl)
                        for n in range(hwl // nfree):
                            pt = ps.tile([P, 512], f32, tag="mm")
                            for k in range(2):
                                nc.tensor.matmul(
                                    out=pt[:, :nfree],
                                    lhsT=f1t[k][:, g * P:(g + 1) * P],
                                    rhs=rhs_src[k][:, n * nfree:(n + 1) * nfree],
                                    start=(k == 0),
                                    stop=(k == 1),
                                )
                            dst = corr[l][:, n * nfree:(n + 1) * nfree]
                            if ncopy % _cmod == _cmod - 1:
                                nc.vector.tensor_copy(out=dst, in_=pt[:, :nfree])
                            else:
                                nc.scalar.copy(out=dst, in_=pt[:, :nfree])
                            ncopy += 1
                    for l in range(NLVL):
                        ofs = G + BASE_L[l] + g * P * HW_L[l]
                        nc.sync.dma_start(
                            slab_d.ap()[ofs:ofs + P * HW_L[l]].rearrange("(p f) -> p f", f=HW_L[l]),
                            corr[l][:],
                        )

                # ---- band gathers + mask + bilinear combine ----
                feats = po.tile([P, NG * FEAT], bf16, tag="feats")
                if _skip_post:
                    nc.vector.memset(feats[:], 0.0)
                for l in range(NLVL) if not _skip_post else []:
                    bl = B_L[l]
                    wl = W_L[l]
                    band = po.tile([P, NG * bl], bf16, tag=f"band_{l}")
                    # HW DGE only honors one offset per partition -> one gather per group
                    if os.environ.get("K_SKIP_GATHER") == "1":
                        nc.vector.memset(band[:], 0.0)
                    else:
                        for g in range(NG):
                            nc.gpsimd.indirect_dma_start(
                                out=band[:, g * bl:(g + 1) * bl],
                                out_offset=None,
                                in_=slab_d.ap()[:, None],
                                in_offset=bass.IndirectOffsetOnAxis(ap=idx_l[l][:, g:g + 1], axis=0),
                                element_offset=G,
                            )
                    if os.environ.get("K_SKIP_CMB") == "1":
                        continue
                    # masked 10x10 patches, contiguous [P, NG*100]
                    pm = po.tile([P, NG * PS * PS], bf16, tag=f"pm_{l}")
                    nc.vector.tensor_tensor(
                        out=pm[:].rearrange("p (g a b) -> p g a b", a=PS, b=PS),
                        in0=_ap_view(band[:], 0, [[bl, NG], [wl, PS], [1, PS]]),
                        in1=m_l[l][:].rearrange("p (g a b) -> p g a b", a=PS, b=PS),
                        op=OP.mult,
                    )
                    # reference: sample (i, j) is at x = cc_x + (i-4), y = cc_y + (j-4),
                    # so out_tap[i, j] = sum_ab w_ab * patch[y=j+a, x=i+b]
                    ov = _ap_view(feats[:], l * S * S, [[FEAT, NG], [S, S], [1, S]])
                    for t, (a, b) in enumerate(((0, 0), (0, 1), (1, 0), (1, 1))):
                        pv = _ap_view(pm[:], a * PS + b, [[PS * PS, NG], [1, S], [PS, S]])
                        wb = w_l[l][t][:, :, None, None].to_broadcast([P, NG, S, S])
                        if t == 0:
                            nc.vector.tensor_tensor(out=ov, in0=pv, in1=wb, op=OP.mult)
                        else:
                            tmp = po.tile([P, NG * S * S], bf16, tag=f"cmb_tmp")
                            tv = tmp[:].rearrange("p (g a b) -> p g a b", a=S, b=S)
                            nc.vector.tensor_tensor(out=tv, in0=pv, in1=wb, op=OP.mult)
                            nc.vector.tensor_tensor(out=ov, in0=ov, in1=tv, op=OP.add)

                nc.sync.dma_start(
                    out_d.ap().rearrange("(g p) f -> p g f", p=P),
                    feats[:].rearrange("p (g f) -> p g f", f=FEAT),
                )

    nc.compile()
    return nc


_NC = None


def _get_nc():
    global _NC
    if _NC is None:
        _NC = build_bass()
    return _NC


def make_in_maps(fmap1, fmap2, centroids_coords):
    in_maps = []
    for core in range(8):
        bi, chunk = divmod(core, 4)
        m0 = chunk * NPIX
        f1 = np.ascontiguousarray(
            fmap1[bi].reshape(C, HW)[:, m0:m0 + NPIX]).astype(ml_dtypes.bfloat16)
        f2 = np.ascontiguousarray(fmap2[bi].reshape(C, HW)).astype(ml_dtypes.bfloat16)
        cc = centroids_coords[bi].reshape(2, HW)[:, m0:m0 + NPIX]
        ccx = np.ascontiguousarray(cc[0].reshape(NG, P).T, dtype=np.float32)  # [p, g]
        ccy = np.ascontiguousarray(cc[1].reshape(NG, P).T, dtype=np.float32)
        in_maps.append({"f1": f1, "f2": f2, "ccx": ccx, "ccy": ccy})
    return in_maps


def assemble(outs):
    """outs: list of 8 arrays [1024, 324] -> [2, 324, 64, 64]"""
    full = np.empty((2, FEAT, 64, 64), dtype=np.float32)
    for bi in range(2):
        feats = np.concatenate(
            [np.asarray(outs[bi * 4 + c]).astype(np.float32) for c in range(4)], axis=0)
        full[bi] = feats.reshape(64, 64, FEAT).transpose(2, 0, 1)
    return full


def kernel(fmap1, fmap2, centroids_coords, trace=False):
    nc = _get_nc()
    in_maps = make_in_maps(fmap1, fmap2, centroids_coords)
    try:
        res = run_bass_kernel_spmd(nc, in_maps, core_ids=list(range(8)), trace=trace)
    except ModuleNotFoundError:
        res = run_bass_kernel_spmd(nc, in_maps, core_ids=list(range(8)), trace=False)
    out = assemble([r["out"] for r in res.results])
    if trace:
        kernel.last_result = res
    return out
